# revision 7
# baseline (speedup 1.0000x reference)
"""Single-head dot-product attention on 8 NeuronCores (Trainium2, Bass/Tile).

Problem (per batch element b, data-parallel over the batch of 8):
    q = x @ Wq; k = x @ Wk; v = x @ Wv          x: [2048, 768], W*: [768, 768]
    out = softmax(q @ k.T / sqrt(768)) @ v

Kernel formulation (per core):
  - The two projection matmuls of the similarity fuse:  q @ k.T =
    x (Wq Wk^T) x^T.  Wqk = Wq Wk^T is batch-independent weight prep and is
    folded on the host (f32, then cast to bf16), so the device computes only
    the four essential data-dependent matmuls:
        tT[d2, q] = sum_d1 Wqk[d1, d2] xT[d1, q]            (t = x Wqk)
        vE[k, 0:769] = [sum_d xT[d, k] Wv[d, :] | 1]        (v = x Wv, plus a
                                                             ones column)
        scoresT[k, q] = sum_d xT[d, k] tT[d, q]             (k on partitions)
        expT = exp(scoresT / sqrt(768))                     (no max-subtraction:
                                                             |scores| <= ~7)
        out_ext[q, 0:769] = sum_k expT[k, q] * vE[k]        (ones column gives
                                                             the denominator)
        out = out_ext[:, :768] * (1 / out_ext[:, 768])
  - All operand layout work (x transpose, bf16 casts) happens on the host, so
    the device does zero PE transposes and zero dtype-cast passes: xT, Wqk,
    Wv arrive in bf16 in exactly the layouts the matmuls consume.
  - Matmul inputs bf16 (fp32 PSUM accumulation); output stored bf16 and
    upcast to f32 on the host.
  - Engine balance: PE does only the 4 matmul families, ACT does exp, DVE
    does PSUM->SBUF copies and the final normalization, GPSIMD the
    ones-column memsets.
"""

import numpy as np

P = 128
S = 2048  # sequence length per core
D = 768   # d_model == q/k/v size
SB = S // P   # 16 s-blocks
DB = D // P   # 6 d-blocks
QSB = 256     # q-superblock (PSUM-bank limited)
NQSB = S // QSB
SCALE = 1.0 / float(np.sqrt(768.0))
N_CORES = 8

_CACHE = {}


def _build_program(reps=1):
    from contextlib import ExitStack

    import concourse.bacc as bacc
    import concourse.mybir as mybir
    import concourse.tile as tile

    f32 = mybir.dt.float32
    bf16 = mybir.dt.bfloat16
    EXP = mybir.ActivationFunctionType.Exp

    nc = bacc.Bacc("TRN2", target_bir_lowering=False, debug=False,
                   num_devices=N_CORES)
    xt_dram = nc.dram_tensor("xt", [D, S], bf16, kind="ExternalInput")
    wqk_dram = nc.dram_tensor("wqk", [D, D], bf16, kind="ExternalInput")
    wv_dram = nc.dram_tensor("wv", [D, D], bf16, kind="ExternalInput")
    y_dram = nc.dram_tensor("y", [S, D], bf16, kind="ExternalOutput")

    with tile.TileContext(nc) as tc, \
         tc.tile_pool(name="persist", bufs=1) as persist, \
         tc.tile_pool(name="wkeep", bufs=1) as wkeep:
      for _rep in range(reps):
        # Parity-alternate the persistent tiles so rep r+1's input DMAs can
        # prefetch into fresh addresses while rep r is still reading its own
        # set (no write-after-read stall at the rep boundary).
        pr = _rep % 2
        top = ExitStack()
        top.__enter__()

        # Persistent bf16 operands for the attention stage. xT is one 3D
        # tile [P, DB, S]; column panels land directly from DMA.
        xT_all = persist.tile([P, DB, S], bf16, tag=f"xT{pr}",
                              name=f"xT_all{pr}")
        xT = [xT_all[:, i, :] for i in range(DB)]
        tT = [persist.tile([P, S], bf16, tag=f"tT{pr}_{i}",
                           name=f"tT{pr}_{i}")
              for i in range(DB)]
        vE = [persist.tile([P, 776], bf16, tag=f"vE{pr}_{i}",
                           name=f"vE{pr}_{i}")
              for i in range(SB)]

        # ---------------- Phase 1: DMA-in + projections --------------------
        with ExitStack() as st:
            pa = st.enter_context(
                tc.tile_pool(name="pa", bufs=3, space="PSUM"))
            pb = st.enter_context(
                tc.tile_pool(name="pb", bufs=2, space="PSUM"))

            wqk = [wkeep.tile([P, D], bf16, tag=f"wqk{pr}_{i}",
                              name=f"wqk{pr}_{i}")
                   for i in range(DB)]
            wv_b = [wkeep.tile([P, D], bf16, tag=f"wv{pr}_{i}",
                               name=f"wv{pr}_{i}")
                    for i in range(DB)]

            # DMA order: Wqk first (gates the first t_proj), then the g0
            # column panel of xT, then Wv, then the remaining panels.
            for db in range(DB):
                nc.sync.dma_start(wqk[db], wqk_dram[db * P:(db + 1) * P, :])

            def x_panel(g):
                for db in range(DB):
                    nc.sync.dma_start(
                        xT_all[:, db, g * 512:(g + 1) * 512],
                        xt_dram[db * P:(db + 1) * P, g * 512:(g + 1) * 512])

            x_panel(0)
            for db in range(DB):
                nc.sync.dma_start(wv_b[db], wv_dram[db * P:(db + 1) * P, :])
            for g in range(1, 4):
                x_panel(g)

            # tT[d2, q] = sum_d1 Wqk[d1, d2] xT[d1, q] for 512-chunk g
            def t_proj(g):
                for d2 in range(DB):
                    pj = pa.tile([P, 512], f32, tag="pa", name="pj")
                    for d1 in range(DB):
                        nc.tensor.matmul(
                            pj, wqk[d1][:, d2 * P:(d2 + 1) * P],
                            xT[d1][:, g * 512:(g + 1) * 512],
                            start=(d1 == 0), stop=(d1 == DB - 1))
                    nc.vector.tensor_copy(
                        tT[d2][:, g * 512:(g + 1) * 512], pj)

            # v[s, e] = sum_d x[s, d] Wv[d, e]; vE = [v | 1]
            def v_proj(sb):
                pv_a = pa.tile([P, 512], f32, tag="pa", name="pv_a")
                pv_b = pb.tile([P, 256], f32, tag="pb", name="pv_b")
                for db in range(DB):
                    lhs = xT[db][:, sb * P:(sb + 1) * P]
                    nc.tensor.matmul(pv_a, lhs, wv_b[db][:, 0:512],
                                     start=(db == 0), stop=(db == DB - 1))
                    nc.tensor.matmul(pv_b, lhs, wv_b[db][:, 512:768],
                                     start=(db == 0), stop=(db == DB - 1))
                nc.vector.tensor_copy(vE[sb][:, 0:512], pv_a)
                nc.vector.tensor_copy(vE[sb][:, 512:768], pv_b)
                nc.gpsimd.memset(vE[sb][:, 768:769], 1.0)

            for g in range(4):
                t_proj(g)
                for sb in range(4 * g, 4 * g + 4):
                    v_proj(sb)

        # ---------------- Attention stage ---------------------------------
        with ExitStack() as st:
            sc_pool = st.enter_context(
                tc.tile_pool(name="sc", bufs=3, space="PSUM"))
            oa_pool = st.enter_context(
                tc.tile_pool(name="oa", bufs=2, space="PSUM"))
            ob_pool = st.enter_context(
                tc.tile_pool(name="ob", bufs=2, space="PSUM"))
            ex_pool = st.enter_context(tc.tile_pool(name="ex", bufs=3))
            yout = st.enter_context(tc.tile_pool(name="yout", bufs=4))

            for qsb in range(NQSB):
                q0 = qsb * QSB
                oa = [oa_pool.tile([P, 512], f32, tag="oa", name=f"oa{qi}")
                      for qi in range(QSB // P)]
                ob = [ob_pool.tile([P, 257], f32, tag="ob", name=f"ob{qi}")
                      for qi in range(QSB // P)]

                def out_mms(ki, ex):
                    for qi in range(QSB // P):
                        lhs = ex[:, qi * P:(qi + 1) * P]
                        nc.tensor.matmul(oa[qi], lhs, vE[ki][:, 0:512],
                                         start=(ki == 0), stop=(ki == SB - 1))
                        nc.tensor.matmul(ob[qi], lhs, vE[ki][:, 512:769],
                                         start=(ki == 0), stop=(ki == SB - 1))

                # Defer each out-matmul two score-steps behind its exp so the
                # PE has ~2us of score work queued while the previous qsb's
                # normalize releases the oa/ob accumulators.
                pend = []
                for ki in range(SB):
                    sc = sc_pool.tile([P, QSB], f32, tag="sc", name="sc")
                    for db in range(DB):
                        nc.tensor.matmul(
                            sc, xT[db][:, ki * P:(ki + 1) * P],
                            tT[db][:, q0:q0 + QSB],
                            start=(db == 0), stop=(db == DB - 1))
                    ex = ex_pool.tile([P, QSB], bf16, tag="ex", name="ex")
                    nc.scalar.activation(ex, sc, EXP, scale=SCALE)
                    pend.append((ki, ex))
                    if len(pend) > 2:
                        out_mms(*pend.pop(0))
                for p in pend:
                    out_mms(*p)

                for qi in range(QSB // P):
                    den = yout.tile([P, 1], f32, tag="den", name="den")
                    nc.vector.reciprocal(den, ob[qi][:, 256:257])
                    yt = yout.tile([P, D], bf16, tag="yt", name="yt")
                    nc.vector.tensor_scalar_mul(yt[:, 0:512], oa[qi], den)
                    nc.vector.tensor_scalar_mul(
                        yt[:, 512:768], ob[qi][:, 0:256], den)
                    r0 = q0 + qi * P
                    nc.sync.dma_start(y_dram[r0:r0 + P, :], yt)

        top.__exit__(None, None, None)

    nc.compile()
    return nc


def _host_prep(inputs):
    """Host-side data marshaling: shard x over cores, fold Wqk = Wq Wk^T,
    cast everything to bf16 in the exact device layouts."""
    import ml_dtypes
    bf16 = ml_dtypes.bfloat16

    x = np.asarray(inputs["inputs"], dtype=np.float32)
    wq = np.asarray(inputs["W_query"], dtype=np.float32)
    wk = np.asarray(inputs["W_key"], dtype=np.float32)
    wv = np.asarray(inputs["W_value"], dtype=np.float32)

    wqk = np.ascontiguousarray(wq @ wk.T).astype(bf16)
    wv_b = np.ascontiguousarray(wv).astype(bf16)
    return {
        "xt": [np.ascontiguousarray(x[b].T).astype(bf16)
               for b in range(N_CORES)],
        "wqk": [wqk] * N_CORES,
        "wv": [wv_b] * N_CORES,
    }


def _get_program():
    if "nc" not in _CACHE:
        _CACHE["nc"] = _build_program()
    return _CACHE["nc"]


def _get_runner():
    """Build the program once and wrap it in a cached sharded jit callable."""
    if "runner" in _CACHE:
        return _CACHE["runner"]

    import jax
    from jax.experimental.shard_map import shard_map
    from jax.sharding import Mesh, PartitionSpec

    import concourse.mybir as mybir
    from concourse.bass2jax import (
        _bass_exec_p,
        install_neuronx_cc_hook,
        partition_id_tensor,
    )

    nc = _get_program()
    install_neuronx_cc_hook()

    partition_name = (nc.partition_id_tensor.name
                      if nc.partition_id_tensor else None)
    in_names, out_names, out_avals, zero_shapes = [], [], [], []
    for alloc in nc.m.functions[0].allocations:
        if not isinstance(alloc, mybir.MemoryLocationSet):
            continue
        name = alloc.memorylocations[0].name
        if alloc.kind == "ExternalInput":
            if name != partition_name:
                in_names.append(name)
        elif alloc.kind == "ExternalOutput":
            out_names.append(name)
            shape = tuple(alloc.tensor_shape)
            dtype = mybir.dt.np(alloc.dtype)
            out_avals.append(jax.core.ShapedArray(shape, dtype))
            zero_shapes.append((shape, dtype))
    n_params = len(in_names)
    all_names = list(in_names) + list(out_names)
    if partition_name is not None:
        all_names.append(partition_name)

    def _body(*args):
        operands = list(args)
        if partition_name is not None:
            operands.append(partition_id_tensor())
        outs = _bass_exec_p.bind(
            *operands,
            out_avals=tuple(out_avals),
            in_names=tuple(all_names),
            out_names=tuple(out_names),
            lowering_input_output_aliases=(),
            sim_require_finite=True,
            sim_require_nnan=True,
            nc=nc,
        )
        return tuple(outs)

    devices = jax.devices()[:N_CORES]
    mesh = Mesh(np.asarray(devices), ("core",))
    n_outs = len(out_names)
    sharded = jax.jit(
        shard_map(_body, mesh=mesh,
                  in_specs=(PartitionSpec("core"),) * (n_params + n_outs),
                  out_specs=(PartitionSpec("core"),) * n_outs,
                  check_rep=False),
        donate_argnums=tuple(range(n_params, n_params + n_outs)),
        keep_unused=True,
    )
    _CACHE["runner"] = (sharded, in_names, zero_shapes)
    return _CACHE["runner"]


def kernel(**inputs):
    sharded, in_names, zero_shapes = _get_runner()

    per_core = _host_prep(inputs)
    concat_in = [np.concatenate(per_core[nm], axis=0) for nm in in_names]
    concat_zeros = [np.zeros((N_CORES * sh[0], *sh[1:]), dt)
                    for sh, dt in zero_shapes]
    outs = sharded(*concat_in, *concat_zeros)
    y = np.asarray(outs[0]).reshape(N_CORES, S, D).astype(np.float32)
    return y
